# revision 1
# baseline (speedup 1.0000x reference)
"""Multi-head attention (B=2, S=2048, D=768, H=12, Dh=64) on 8 Trainium2 cores.

Sharding: core c handles batch b=c//4 and head-group g=c%4 (3 heads each).
Host sums the 4 partial y's per batch and applies all output biases.

v2 structural changes vs baseline:
  - V computed directly in [key, dh] layout (lhsT=qT keys), no PE transposes.
  - PV weights are 128 columns wide (V|ones|junk) to trigger FWL; junk rows
    64-127 of pv are ignored.
  - V bias folded into the host-side output bias (W_out @ b_v), K/Q biases
    stay on-chip.
  - normalization: reciprocal on the [1, 3*512] den row, partition_broadcast
    on GpSimd, O-mul on GpSimd; no ones-matmul broadcast, no [64,512]
    reciprocals.
  - yproj: heads 0+1 stacked on 128 partitions (one 128-contraction matmul)
    + h2 matmul; accumulates in one PSUM bank, DMA'd straight from PSUM
    (b_out applied on host).
  - projection bias-adds merged to one [128,512] DVE op + 4 dup DMAs.
  - K/Q/V projections fully interleaved into the attention grp loop.
"""

import math

import numpy as np
import ml_dtypes

import concourse.bass as bass
import concourse.mybir as mybir
import concourse.tile as tile
from concourse import bacc, bass_utils
from concourse.bass import ts, ds

B, S, D = 2, 2048, 768
H, DH = 12, 64
NCORES = 8
HPC = 3
SCALE = 1.0 / math.sqrt(DH)

f32 = mybir.dt.float32
bf16 = mybir.dt.bfloat16
BF16NP = ml_dtypes.bfloat16

QC = 512
NQC = S // QC
NKB = S // 128
NGRP = NKB // 2

# (qc, grp, h) exp tiles computed on DVE via Schraudolph instead of ACT
OFFLOAD = {(2, g, 1) for g in range(1, 8)} | {(3, g, 1) for g in range(1, 8)}


def build_program():
    nc = bacc.Bacc("TRN2", target_bir_lowering=False, debug=False)
    qT_d = nc.dram_tensor("qT", [D, S], bf16, kind="ExternalInput").ap()
    wt_d = nc.dram_tensor("wt", [D, 576], bf16, kind="ExternalInput").ap()
    bias_d = nc.dram_tensor("biasqk", [128, 3], f32, kind="ExternalInput").ap()
    wo01_d = nc.dram_tensor("wo01", [128, D], bf16, kind="ExternalInput").ap()
    wo2_d = nc.dram_tensor("wo2", [64, D], bf16, kind="ExternalInput").ap()
    yT_d = nc.dram_tensor("yT", [D, S], bf16, kind="ExternalOutput").ap()

    with tile.TileContext(nc) as tc:
        emit(tc, nc, qT_d, wt_d, bias_d, wo01_d, wo2_d, yT_d)
    nc.compile()
    return nc


def emit(tc, nc, qT_d, wt_d, bias_d, wo01_d, wo2_d, yT_d):
    Exp = mybir.ActivationFunctionType.Exp
    yT_r = yT_d.rearrange("(o p) s -> p o s", p=128)
    qT_r = qT_d.rearrange("(o p) s -> p o s", p=128)

    import contextlib
    with contextlib.ExitStack() as octx:
        cpool = octx.enter_context(tc.tile_pool(name="cpool", bufs=1))

        scratch = cpool.tile([1, 16], f32, name="scratch")
        bias_sb = cpool.tile([128, 3], f32, name="bias_sb")
        nc.sync.dma_start(bias_sb, bias_d)
        # early activation-table load (Exp), before any real dependency
        nc.scalar.activation(scratch, scratch, Exp)
        ones1 = cpool.tile([1, 64], bf16, name="ones1")
        nc.vector.memset(ones1, 1.0)
        # Schraudolph exp-approx constants (bf16 bit space):
        # i16 = x*128*log2(e) + 128*(127-c); bitcast int16 -> bf16 ~= exp(x)
        bconst = cpool.tile([128, 1], f32, name="bconst")
        nc.vector.memset(bconst, 16249.6665)

        # per-head Q/K, duplicated on both partition halves for row packing
        Qd = [cpool.tile([128, S], bf16, name=f"Qd{h}") for h in range(HPC)]
        Kd = [cpool.tile([128, S], bf16, name=f"Kd{h}") for h in range(HPC)]
        # V in [key, col] layout; per head h cols 65h..65h+63 = V_h,
        # col 65h+64 = ones (denominator).
        V_sb = cpool.tile([128, NKB, 195], bf16, name="V_sb")
        nc.gpsimd.memset(V_sb, 0.0)
        for h in range(HPC):
            nc.vector.memset(V_sb[:, :, 65 * h + 64: 65 * h + 65], 1.0)

        ppool = octx.enter_context(tc.tile_pool(name="prep", bufs=1))

        qTc_tiles = {}

        def get_qTc(c):
            if c not in qTc_tiles:
                t = ppool.tile([128, 6, QC], bf16, name=f"qTc{c}",
                               tag="qTc", bufs=3)
                for cc in range(6):
                    nc.sync.dma_start(t[:, cc, :],
                                      qT_r[:, cc, ds(c * QC, QC)])
                qTc_tiles[c] = t
            return qTc_tiles[c]

        wt_sb = cpool.tile([128, 6, 576], bf16, name="wt_sb")
        wt_r = wt_d.rearrange("(o p) m -> p o m", p=128)
        # interleave first qT chunk with weights so prologue matmuls can
        # start as soon as their cc-slice has landed
        t0 = ppool.tile([128, 6, QC], bf16, name="qTc0", tag="qTc", bufs=3)
        for cc in range(6):
            nc.sync.dma_start(t0[:, cc, :], qT_r[:, cc, ds(0, QC)])
            nc.sync.dma_start(wt_sb[:, cc, :], wt_r[:, cc, :])
        qTc_tiles[0] = t0
        wo01_sb = cpool.tile([128, D], bf16, name="wo01_sb")
        wo2_sb = cpool.tile([64, D], bf16, name="wo2_sb")

        with tc.tile_pool(name="attn", bufs=2) as apool, \
             tc.tile_pool(name="ps_s", bufs=2, space="PSUM") as psS, \
             tc.tile_pool(name="ps_pv", bufs=1, space="PSUM") as psPV, \
             tc.tile_pool(name="ps_aux", bufs=1, space="PSUM") as psA:

            # ---- projection unit: one M-block x one 512-col chunk ----
            # row blocks 0:[Qh0 Qh1] 1:[Qh2 Kh2] 2:[Kh0 Kh1]
            DSTS = {0: (Qd[0], Qd[1]), 1: (Qd[2], Kd[2]), 2: (Kd[0], Kd[1])}

            def emit_proj(mi, c, pool=None):
                qTc = get_qTc(c)
                sl = ds(c * QC, QC)
                if pool is None:
                    ps = psA.tile([128, QC], f32, name="ps", tag="aux")
                else:
                    ps = pool.tile([128, 2 * QC], f32, name="ps",
                                   tag="psc")[:, 0:QC]
                for cc in range(6):
                    nc.tensor.matmul(ps, lhsT=wt_sb[:, cc, ds(mi * 128, 128)],
                                     rhs=qTc[:, cc, :],
                                     start=(cc == 0), stop=(cc == 5))
                d0, d1 = DSTS[mi]
                if pool is not None:
                    # prologue: write the halves each mm consumer needs first
                    # directly, dup to the other half off the critical path
                    nc.vector.tensor_add(
                        d0[0:64, sl], ps[0:64],
                        bias_sb[0:64, mi:mi + 1].to_broadcast((64, QC)))
                    nc.vector.tensor_add(
                        d1[64:128, sl], ps[64:128],
                        bias_sb[64:128, mi:mi + 1].to_broadcast((64, QC)))
                    nc.sync.dma_start(d0[64:128, sl], d0[0:64, sl])
                    nc.sync.dma_start(d1[0:64, sl], d1[64:128, sl])
                    return
                tmp = apool.tile([128, QC], bf16, name="tmp", tag="tmp")
                nc.vector.tensor_add(
                    tmp, ps,
                    bias_sb[:, mi:mi + 1].to_broadcast((128, QC)))
                nc.sync.dma_start(d0[0:64, sl], tmp[0:64])
                nc.sync.dma_start(d0[64:128, sl], tmp[0:64])
                nc.sync.dma_start(d1[0:64, sl], tmp[64:128])
                nc.sync.dma_start(d1[64:128, sl], tmp[64:128])

            # ---- direct-V unit: one 128-key block ----
            def emit_v(kb):
                qTc = get_qTc(kb // 4)
                ps = psA.tile([128, QC], f32, name="ps", tag="aux")
                for cc in range(6):
                    nc.tensor.matmul(ps[:, 0:192],
                                     lhsT=qTc[:, cc, ds((kb % 4) * 128, 128)],
                                     rhs=wt_sb[:, cc, ds(384, 192)],
                                     start=(cc == 0), stop=(cc == 5))
                for h in range(HPC):
                    nc.vector.tensor_copy(
                        V_sb[:, kb, ds(65 * h, 64)],
                        ps[:, ds(64 * h, 64)])

            # ---- prologue: first chunk of K and Q (via psc bufs) ----
            emit_proj(2, 0, pool=psS)
            emit_proj(0, 0, pool=psS)
            emit_proj(1, 0, pool=psS)
            nc.sync.dma_start(wo01_sb, wo01_d)
            nc.sync.dma_start(wo2_sb, wo2_d)

            deferred = [
                [("V", 0), ("V", 1), ("V", 2), ("V", 3)],      # after grp0
                [("P", 2, 1), ("P", 1, 1)],                    # grp1
                [("V", 4), ("V", 5)],                          # grp2
                [("P", 2, 2), ("P", 1, 2)],                    # grp3
                [("V", 6), ("V", 7), ("V", 8), ("V", 9)],      # grp4
                [("P", 2, 3), ("P", 1, 3)],                    # grp5
                [("V", 10), ("V", 11), ("V", 12), ("V", 13)],  # grp6
                [("P", 0, 1), ("V", 14), ("V", 15)],           # grp7
                [], [], [], [],                                # qc1 grp0-3
                [("P", 0, 2)],                                 # qc1 grp4
                [], [], [], [],                                # qc1 g5-7, qc2 g0
                [("P", 0, 3)],                                 # qc2 grp1
            ]

            def pump_deferred(slot):
                if slot < len(deferred):
                    for unit in deferred[slot]:
                        if unit[0] == "V":
                            emit_v(unit[1])
                        else:
                            emit_proj(unit[1], unit[2])

            # ---- norm + output projection for one q-chunk ----
            def norm_steps(pv, qoff, W, last):
                qsl = ds(qoff, W)
                pvc = apool.tile([64, HPC, QC], f32, name="pvc", tag="pvc")
                denb = apool.tile([1, HPC, QC], bf16, name="denb", tag="denb")
                recs = apool.tile([64, HPC, QC], f32, name="recs", tag="recs")
                Ost = apool.tile([128, QC], bf16, name="Ost", tag="Ost")
                Oh1 = apool.tile([64, QC], bf16, name="Oh1", tag="Oh1")
                Oh2 = apool.tile([64, QC], bf16, name="Oh2", tag="Oh2")
                mul_eng = nc.vector if last else nc.gpsimd

                def brc(h):
                    bcD = psA.tile([128, QC], f32, name="bcD", tag="aux")
                    nc.tensor.matmul(bcD[0:64, 0:W], lhsT=ones1,
                                     rhs=denb[:, h, 0:W])
                    nc.vector.reciprocal_approx_fast(recs[:, h, 0:W],
                                                     bcD[0:64, 0:W])

                Odst = (Ost[0:64], Oh1, Oh2)
                steps = []
                for h in range(HPC):
                    steps.append(lambda h=h: nc.vector.tensor_copy(
                        denb[:, h, 0:W], pv[h][64:65, 0:W]))
                    steps.append(lambda h=h: nc.vector.tensor_copy(
                        pvc[:, h, 0:W], pv[h][0:64, 0:W]))
                    steps.append(lambda h=h: brc(h))
                    steps.append(lambda h=h: mul_eng.tensor_mul(
                        Odst[h][:, 0:W], pvc[:, h, 0:W], recs[:, h, 0:W]))

                def oh1_move():
                    nc.sync.dma_start(Ost[64:128, 0:W], Oh1[:, 0:W])
                steps.insert(8, oh1_move)

                def proj_jb(jb):
                    if last and jb % 2 == 1:
                        yps = psPV.tile([128, QC], f32, name="ypv", tag="pv0")
                    else:
                        yps = psA.tile([128, QC], f32, name="yps", tag="aux")
                    nc.tensor.matmul(yps[:, 0:W], lhsT=wo01_sb[:, ts(jb, 128)],
                                     rhs=Ost[:, 0:W], start=True, stop=False)
                    nc.tensor.matmul(yps[:, 0:W], lhsT=wo2_sb[:, ts(jb, 128)],
                                     rhs=Oh2[:, 0:W], start=False, stop=True)
                    ysb = apool.tile([128, QC], bf16, name="ysb", tag="ysb")
                    nc.vector.tensor_copy(ysb[:, 0:W], yps[:, 0:W])
                    nc.sync.dma_start(yT_r[:, jb, qsl], ysb[:, 0:W])

                steps += [lambda jb=jb: proj_jb(jb) for jb in range(6)]
                return steps

            pending = []
            CHUNKS = [(i * QC, QC) for i in range(NQC)]
            for ci, (qoff, W) in enumerate(CHUNKS):
                qsl = ds(qoff, W)
                slot0 = 8 * ci
                last = ci == len(CHUNKS) - 1
                attn = [apool.tile([128, NKB * QC], bf16,
                                   name=f"attn{h}", tag=f"attn{h}")
                        for h in range(HPC)]
                pv = [psPV.tile([128, QC], f32, name=f"pv{h}", tag=f"pv{h}")
                      for h in range(HPC)]

                def emit_pv(g):
                    for h in range(HPC):
                        for kb in (2 * g, 2 * g + 1):
                            nc.tensor.matmul(
                                pv[h][0:65, 0:W],
                                lhsT=V_sb[:, kb, ds(65 * h, 65)],
                                rhs=attn[h][:, kb * W:(kb + 1) * W],
                                start=(kb == 0), stop=(kb == NKB - 1),
                                skip_group_check=True)

                for grp in range(NGRP):
                    kb0, kb1 = 2 * grp, 2 * grp + 1
                    for h in range(HPC):
                        hi = 0 if (ci == 0 and grp == 0) else 64
                        psc = psS.tile([128, 2 * QC], f32, name="psc",
                                       tag="psc")
                        # second matmul lands bank-aligned at QC
                        nc.tensor.matmul(psc[:, 0:W],
                                         lhsT=Kd[h][0:64, ts(kb0, 128)],
                                         rhs=Qd[h][0:64, qsl])
                        nc.tensor.matmul(psc[:, QC:QC + W],
                                         lhsT=Kd[h][hi:hi + 64, ts(kb1, 128)],
                                         rhs=Qd[h][hi:hi + 64, qsl])
                        ob = attn[h][:, grp * 2 * W:(grp + 1) * 2 * W]
                        if W == QC:
                            if (ci, grp, h) in OFFLOAD:
                                nc.vector.scalar_tensor_tensor(
                                    ob.bitcast(mybir.dt.int16),
                                    psc, 184.6650292,
                                    bconst.to_broadcast((128, 2 * W)),
                                    mybir.AluOpType.mult,
                                    mybir.AluOpType.add)
                            else:
                                nc.scalar.activation(ob, psc, Exp)
                        elif (ci, grp, h) in OFFLOAD:
                            for i2 in range(2):
                                nc.vector.scalar_tensor_tensor(
                                    ob[:, i2 * W:(i2 + 1) * W]
                                    .bitcast(mybir.dt.int16),
                                    psc[:, i2 * QC:i2 * QC + W],
                                    184.6650292,
                                    bconst.to_broadcast((128, W)),
                                    mybir.AluOpType.mult,
                                    mybir.AluOpType.add)
                        else:
                            for i2 in range(2):
                                nc.scalar.activation(
                                    ob[:, i2 * W:(i2 + 1) * W],
                                    psc[:, i2 * QC:i2 * QC + W], Exp)
                    pump_deferred(slot0 + grp)
                    if last:
                        emit_pv(grp)
                    elif grp > 0:
                        emit_pv(grp - 1)
                    for _ in range(3):
                        if pending:
                            pending.pop(0)()
                if not last:
                    emit_pv(NGRP - 1)
                while pending:
                    pending.pop(0)()
                pending = norm_steps(pv, qoff, W, last)
            while pending:
                pending.pop(0)()


# ---------------------------------------------------------------------------
# host side
# ---------------------------------------------------------------------------

def make_core_inputs(q, W_qkv, b_qkv, W_out, b_out):
    q = np.asarray(q, np.float32)
    W_qkv = np.asarray(W_qkv, np.float32)
    b_qkv = np.asarray(b_qkv, np.float32)
    W_out = np.asarray(W_out, np.float32)

    Wq, Wk, Wv = W_qkv[0:D], W_qkv[D:2 * D], W_qkv[2 * D:3 * D]
    bq, bk = b_qkv[0:D], b_qkv[D:2 * D]

    def hrows(W, h):
        return W[h * DH:(h + 1) * DH]

    def hbias(bvec, h):
        return bvec[h * DH:(h + 1) * DH]

    in_maps = []
    for c in range(NCORES):
        b = c // 4
        g = c % 4
        h0, h1, h2 = 3 * g, 3 * g + 1, 3 * g + 2

        qT = np.ascontiguousarray(q[b].T).astype(BF16NP)

        wt = np.concatenate([
            hrows(Wq, h0) * SCALE, hrows(Wq, h1) * SCALE,
            hrows(Wq, h2) * SCALE, hrows(Wk, h2),
            hrows(Wk, h0), hrows(Wk, h1),
            hrows(Wv, h0), hrows(Wv, h1), hrows(Wv, h2),
        ], axis=0)
        wt = np.ascontiguousarray(wt.T).astype(BF16NP)

        biasqk = np.stack([
            np.concatenate([hbias(bq, h0), hbias(bq, h1)]) * SCALE,
            np.concatenate([hbias(bq, h2) * SCALE, hbias(bk, h2)]),
            np.concatenate([hbias(bk, h0), hbias(bk, h1)]),
        ], axis=1).astype(np.float32)

        wo01 = np.concatenate([
            W_out[:, h0 * DH:(h0 + 1) * DH].T,
            W_out[:, h1 * DH:(h1 + 1) * DH].T,
        ], axis=0)
        wo01 = np.ascontiguousarray(wo01).astype(BF16NP)
        wo2 = np.ascontiguousarray(
            W_out[:, h2 * DH:(h2 + 1) * DH].T).astype(BF16NP)

        in_maps.append({
            "qT": qT, "wt": wt, "biasqk": biasqk,
            "wo01": wo01, "wo2": wo2,
        })
    return in_maps


_NC = None


def _get_nc():
    global _NC
    if _NC is None:
        _NC = build_program()
    return _NC


def kernel(q, k, v, W_qkv, b_qkv, W_out, b_out, _trace=False):
    nc = _get_nc()
    in_maps = make_core_inputs(q, W_qkv, b_qkv, W_out, b_out)
    res = bass_utils.run_bass_kernel_spmd(
        nc, in_maps, core_ids=list(range(NCORES)), trace=_trace)
    kernel.last_result = res
    W_out = np.asarray(W_out, np.float32)
    bv = np.asarray(b_qkv, np.float32)[2 * D:3 * D]
    bias = np.asarray(b_out, np.float32) + W_out @ bv
    y = np.empty((B, S, D), np.float32)
    for b in range(B):
        acc = res.results[4 * b]["yT"].astype(np.float32)
        for g in range(1, 4):
            acc = acc + res.results[4 * b + g]["yT"]
        y[b] = acc.T + bias
    return y

